# revision 7
# baseline (speedup 1.0000x reference)
"""NARX-RNN (nn_Narxnn) Trainium2 Bass kernel.

Model (reference): for each window t in [0, 508) and each grid g:
    h_0 = 0
    for d in 1..4:  u = relu(x[t+d-1, g] @ W_in + b_in)
                    h_d = tanh(u @ W_xh + h_{d-1} @ W_hh + b_h)
    out[t+4, g, 0] = h_4 @ W_out + b_out

Sharding: pure data-parallel over ngrid (2048 -> 8 cores x 256 grids).

Per-core layout trick: host re-packs x so SBUF partitions carry
(8 grids x 16 features) and the free axis carries time. A single
block-diagonal W_in matmul per 32-row strip then produces U directly in
[2 grids x 64 hidden, time] layout -- no on-device transposes. The
recurrence runs with time as the matmul free axis; the lag-window shift
is just an AP offset into U. All matmuls use float32r (full PE rate).
"""

import os
import sys

for _p in ("/opt/trn_rl_repo", "/root/.axon_site/_ro/trn_rl_repo"):
    if os.path.isdir(_p) and _p not in sys.path:
        sys.path.insert(0, _p)

import numpy as np

import concourse.bass as bass  # noqa: E402
import concourse.mybir as mybir  # noqa: E402
import concourse.tile as tile  # noqa: E402
from concourse import bacc  # noqa: E402
from concourse.bass_utils import run_bass_kernel_spmd  # noqa: E402

NT, NGRID, NX = 512, 2048, 16
NY, HID, D = 1, 64, 4
N_CORES = 8
G_CORE = NGRID // N_CORES  # 256 grids per core
NOCT = G_CORE // 8         # 32 octets of 8 grids
TPAD = 520                 # 512 time steps + 8 zero pad (chain reads t+3 up to 514)
TW = 512                   # window columns computed per pair (508 real + 4 discarded)

f32 = mybir.dt.float32
f32r = mybir.dt.float32r

_NC_CACHE = None


def _build_nc():
    nc = bacc.Bacc(None, target_bir_lowering=False, debug=False)

    xr_d = nc.declare_dram_parameter("xr", [128, NOCT, TPAD], f32r, isOutput=False)
    wu4_d = nc.declare_dram_parameter("wu4", [128, 128], f32r, isOutput=False)
    wxh2_d = nc.declare_dram_parameter("wxh2", [128, 128], f32r, isOutput=False)
    whh2_d = nc.declare_dram_parameter("whh2", [128, 128], f32r, isOutput=False)
    woutv_d = nc.declare_dram_parameter("woutv", [128, 4, 8], f32r, isOutput=False)
    bin2_d = nc.declare_dram_parameter("bin2", [128, 1], f32, isOutput=False)
    bh2_d = nc.declare_dram_parameter("bh2", [128, 1], f32, isOutput=False)
    bout_d = nc.declare_dram_parameter("bout", [128, 1], f32, isOutput=False)
    y_d = nc.declare_dram_parameter("y", [8, NOCT, TW], f32, isOutput=True)

    Tanh = mybir.ActivationFunctionType.Tanh

    tc_cm = tile.TileContext(nc)
    tc = tc_cm.__enter__()
    try:
        with (
            tc.tile_pool(name="consts", bufs=1) as consts,
            tc.tile_pool(name="xin", bufs=3) as xin,
            tc.tile_pool(name="upool", bufs=8) as upool,
            tc.tile_pool(name="hpool", bufs=6) as hpool,
            tc.tile_pool(name="zq", bufs=2, space="PSUM") as zpool,
            tc.tile_pool(name="ysb", bufs=3) as ypool,
        ):
            wu4 = consts.tile([128, 128], f32r)
            nc.sync.dma_start(out=wu4, in_=wu4_d[:, :])
            wxh2 = consts.tile([128, 128], f32r)
            nc.sync.dma_start(out=wxh2, in_=wxh2_d[:, :])
            whh2 = consts.tile([128, 128], f32r)
            nc.sync.dma_start(out=whh2, in_=whh2_d[:, :])
            woutv = consts.tile([128, 4, 8], f32r)
            nc.sync.dma_start(out=woutv, in_=woutv_d[:, :, :])
            bin2 = consts.tile([128, 1], f32)
            nc.sync.dma_start(out=bin2, in_=bin2_d[:, :])
            bh2 = consts.tile([128, 1], f32)
            nc.sync.dma_start(out=bh2, in_=bh2_d[:, :])
            bout = consts.tile([128, 1], f32)
            nc.sync.dma_start(out=bout, in_=bout_d[:, :])

            for o in range(NOCT):
                xt = xin.tile([128, TPAD], f32r, tag="xt")
                nc.sync.dma_start(out=xt, in_=xr_d[:, o, :])

                zq = zpool.tile([128, 2048], f32, tag="zq")
                Us = [
                    upool.tile([128, TPAD], f32r, tag="U", name=f"U{o}_{s}")
                    for s in range(4)
                ]

                # U = relu(x @ W_in + b_in), produced as [2g x 64h, t] per
                # strip via block-diagonal weights. Two rounds of 260 cols
                # (bank holds 512; U spans 520).
                for r in range(2):
                    c0 = 260 * r
                    for s in range(4):
                        nc.tensor.matmul(
                            out=zq[:, 512 * s : 512 * s + 260],
                            lhsT=wu4[32 * s : 32 * s + 32, :],
                            rhs=xt[32 * s : 32 * s + 32, c0 : c0 + 260],
                            start=True,
                            stop=True,
                            tile_position=(32 * s, 0),
                        )
                    for s in range(4):
                        nc.vector.tensor_scalar(
                            out=Us[s][:, c0 : c0 + 260],
                            in0=zq[:, 512 * s : 512 * s + 260],
                            scalar1=bin2,
                            scalar2=0.0,
                            op0=mybir.AluOpType.add,
                            op1=mybir.AluOpType.max,
                        )

                # Recurrence: z_d = u_{t+d-1} @ W_xh (+ h_{d-1} @ W_hh),
                # h_d = tanh(z_d + b_h). Even/odd grid of each pair run on
                # disjoint PE quadrants (rows 0-63 / 64-127).
                hprev = None
                for d in range(1, 5):
                    off = d - 1
                    for p in range(4):
                        nc.tensor.matmul(
                            out=zq[:, 512 * p : 512 * p + TW],
                            lhsT=wxh2,
                            rhs=Us[p][:, off : off + TW],
                            start=True,
                            stop=(d == 1),
                        )
                        if d > 1:
                            nc.tensor.matmul(
                                out=zq[:, 512 * p : 512 * p + TW],
                                lhsT=whh2,
                                rhs=hprev[:, 512 * p : 512 * p + TW],
                                start=False,
                                stop=True,
                            )
                    h = hpool.tile([128, 2048], f32r, tag="h")
                    nc.scalar.activation(out=h, in_=zq[:, :], func=Tanh, bias=bh2)
                    hprev = h

                # y = h4 @ W_out + b_out: one-hot-column W_out variants place
                # each grid's output on its own PSUM row (rows 0-7 of bank 0).
                for p in range(4):
                    nc.tensor.matmul(
                        out=zq[0:8, 0:TW],
                        lhsT=woutv[:, p, :],
                        rhs=hprev[:, 512 * p : 512 * p + TW],
                        start=(p == 0),
                        stop=(p == 3),
                    )
                ysb = ypool.tile([8, TW], f32, tag="ysb")
                nc.vector.tensor_scalar(
                    out=ysb,
                    in0=zq[0:8, 0:TW],
                    scalar1=bout[0:8],
                    scalar2=0.0,
                    op0=mybir.AluOpType.add,
                    op1=mybir.AluOpType.add,
                )
                nc.sync.dma_start(out=y_d[:, o, :], in_=ysb)

        sched_state, _ = tc.schedule_and_allocate()
        try:
            disp = sched_state.get_inst_dispatch_ns()
            vals = list(disp.values()) if hasattr(disp, "values") else list(disp)
            nc._pred_ns = max(t.dispatch_time_ns + t.cost_ns for t in vals) if vals else None
            busy = {}
            for t in vals:
                busy[str(t.engine)] = busy.get(str(t.engine), 0.0) + t.cost_ns
            nc._pred_busy = busy
        except Exception as e:  # best-effort metric only
            nc._pred_ns = None
            nc._pred_err = repr(e)
    finally:
        tc_cm.__exit__(None, None, None)

    nc.finalize()
    return nc


def _get_nc():
    global _NC_CACHE
    if _NC_CACHE is None:
        _NC_CACHE = _build_nc()
    return _NC_CACHE


def _prep_weights(W_in, b_in, W_xh, W_hh, b_h, W_out, b_out):
    wu4 = np.zeros((128, 128), np.float32)
    for s in range(4):
        wu4[32 * s : 32 * s + 16, 0:64] = W_in
        wu4[32 * s + 16 : 32 * s + 32, 64:128] = W_in
    wxh2 = np.zeros((128, 128), np.float32)
    wxh2[0:64, 0:64] = W_xh
    wxh2[64:128, 64:128] = W_xh
    whh2 = np.zeros((128, 128), np.float32)
    whh2[0:64, 0:64] = W_hh
    whh2[64:128, 64:128] = W_hh
    woutv = np.zeros((128, 4, 8), np.float32)
    for p in range(4):
        woutv[0:64, p, 2 * p] = W_out[:, 0]
        woutv[64:128, p, 2 * p + 1] = W_out[:, 0]
    bin2 = np.concatenate([b_in, b_in]).reshape(128, 1).astype(np.float32)
    bh2 = np.concatenate([b_h, b_h]).reshape(128, 1).astype(np.float32)
    bout = np.full((128, 1), np.float32(b_out[0]), np.float32)
    return dict(wu4=wu4, wxh2=wxh2, whh2=whh2, woutv=woutv, bin2=bin2, bh2=bh2, bout=bout)


def _prep_x_core(x_c):
    # x_c: [NT, G_CORE, NX] -> xr [128, NOCT, TPAD] with
    # xr[16*b + f, o, t] = x_c[t, 8*o + b, f]
    xr = np.zeros((128, NOCT, TPAD), np.float32)
    t = x_c.transpose(1, 2, 0)                   # [g, f, t]
    t = t.reshape(NOCT, 8, NX, NT)               # [o, b, f, t]
    t = t.transpose(1, 2, 0, 3).reshape(128, NOCT, NT)
    xr[:, :, :NT] = t
    return xr


def kernel(x, W_in, b_in, W_xh, W_hh, b_h, W_out, b_out, _trace=False):
    nc = _get_nc()
    w = _prep_weights(
        np.asarray(W_in, np.float32), np.asarray(b_in, np.float32),
        np.asarray(W_xh, np.float32), np.asarray(W_hh, np.float32),
        np.asarray(b_h, np.float32), np.asarray(W_out, np.float32),
        np.asarray(b_out, np.float32),
    )
    x = np.asarray(x, np.float32)
    in_maps = []
    for c in range(N_CORES):
        x_c = x[:, c * G_CORE : (c + 1) * G_CORE, :]
        in_maps.append({"xr": _prep_x_core(x_c), **w})

    res = run_bass_kernel_spmd(nc, in_maps, list(range(N_CORES)), trace=_trace)

    out = np.zeros((NT, NGRID, NY), np.float32)
    T = NT - D
    for c in range(N_CORES):
        y_c = res.results[c]["y"]                # [8, NOCT, TW]
        y_c = y_c[:, :, :T]                      # drop garbage windows
        y_c = y_c.transpose(2, 1, 0).reshape(T, G_CORE)  # [t, g]
        out[D:, c * G_CORE : (c + 1) * G_CORE, 0] = y_c
    if _trace:
        kernel.last_result = res
    return out


# revision 10
# speedup vs baseline: 1.3069x; 1.3069x over previous
"""NARX-RNN (nn_Narxnn) Trainium2 Bass kernel.

Model (reference): for each window t in [0, 508) and each grid g:
    h_0 = 0
    for d in 1..4:  u = relu(x[t+d-1, g] @ W_in + b_in)
                    h_d = tanh(u @ W_xh + h_{d-1} @ W_hh + b_h)
    out[t+4, g, 0] = h_4 @ W_out + b_out

Sharding: pure data-parallel over ngrid (2048 -> 8 cores x 256 grids).

Per-core layout trick: host re-packs x so SBUF partitions carry
(8 grids x 16 features) and the free axis carries time. A single
block-diagonal W_in matmul per 32-row strip then produces U directly in
[2 grids x 64 hidden, time] layout -- no on-device transposes. The
recurrence runs with time as the matmul free axis; the lag-window shift
is just an AP offset into U. All matmuls use float32r (full PE rate).
"""

import os
import sys

for _p in ("/opt/trn_rl_repo", "/root/.axon_site/_ro/trn_rl_repo"):
    if os.path.isdir(_p) and _p not in sys.path:
        sys.path.insert(0, _p)

import numpy as np

import concourse.bass as bass  # noqa: E402
import concourse.mybir as mybir  # noqa: E402
import concourse.tile as tile  # noqa: E402
from concourse import bacc  # noqa: E402
from concourse.bass_utils import run_bass_kernel_spmd  # noqa: E402

NT, NGRID, NX = 512, 2048, 16
NY, HID, D = 1, 64, 4
N_CORES = 8
G_CORE = NGRID // N_CORES  # 256 grids per core
NOCT = G_CORE // 8         # 32 octets of 8 grids
TPAD = 512                 # stored time steps per grid (no pad; tail windows are discarded)
USTRIDE = 520              # U tile stride per pair; cols 512+ are junk feeding discarded windows
TW = 512                   # window columns computed per pair (508 real + 4 discarded)

f32 = mybir.dt.float32
f32r = mybir.dt.float32r

_NC_CACHE = None


def _build_nc():
    nc = bacc.Bacc(None, target_bir_lowering=False, debug=False)

    xr_d = nc.declare_dram_parameter("xr", [128, NOCT, TPAD], f32r, isOutput=False)
    wu4_d = nc.declare_dram_parameter("wu4", [128, 128], f32r, isOutput=False)
    wxh2_d = nc.declare_dram_parameter("wxh2", [128, 128], f32r, isOutput=False)
    whh2_d = nc.declare_dram_parameter("whh2", [128, 128], f32r, isOutput=False)
    woutv_d = nc.declare_dram_parameter("woutv", [128, 4, 8], f32r, isOutput=False)
    bin2_d = nc.declare_dram_parameter("bin2", [128, 1], f32, isOutput=False)
    bh2_d = nc.declare_dram_parameter("bh2", [128, 1], f32, isOutput=False)
    bout_d = nc.declare_dram_parameter("bout", [128, 1], f32, isOutput=False)
    y_d = nc.declare_dram_parameter("y", [8, NOCT, TW], f32, isOutput=True)

    Tanh = mybir.ActivationFunctionType.Tanh

    tc_cm = tile.TileContext(nc)
    tc = tc_cm.__enter__()
    try:
        with (
            tc.tile_pool(name="consts", bufs=1) as consts,
            tc.tile_pool(name="xin", bufs=4) as xin,
            tc.tile_pool(name="upool", bufs=3) as upool,
            tc.tile_pool(name="hpool", bufs=8) as hpool,
            tc.tile_pool(name="zq", bufs=2, space="PSUM") as zpool,
            tc.tile_pool(name="ysb", bufs=4) as ypool,
        ):
            wu4 = consts.tile([128, 128], f32r)
            nc.sync.dma_start(out=wu4, in_=wu4_d[:, :])
            wxh2 = consts.tile([128, 128], f32r)
            nc.sync.dma_start(out=wxh2, in_=wxh2_d[:, :])
            whh2 = consts.tile([128, 128], f32r)
            nc.sync.dma_start(out=whh2, in_=whh2_d[:, :])
            woutv = consts.tile([128, 4, 8], f32r)
            nc.sync.dma_start(out=woutv, in_=woutv_d[:, :, :])
            bin2 = consts.tile([128, 1], f32)
            nc.sync.dma_start(out=bin2, in_=bin2_d[:, :])
            bh2 = consts.tile([128, 1], f32)
            nc.sync.dma_start(out=bh2, in_=bh2_d[:, :])
            bout = consts.tile([128, 1], f32)
            nc.sync.dma_start(out=bout, in_=bout_d[:, :])

            def emit_u_phase(o):
                xt = xin.tile([128, TPAD], f32r, tag="xt", name=f"xt{o}")
                nc.sync.dma_start(out=xt, in_=xr_d[:, o, :])
                zq = zpool.tile([128, 2048], f32, tag="zq", name=f"zq{o}")
                ua = upool.tile([128, 4 * USTRIDE], f32r, tag="U", name=f"U{o}")
                for st in range(4):
                    nc.tensor.matmul(
                        out=zq[:, 512 * st : 512 * st + 512],
                        lhsT=wu4[32 * st : 32 * st + 32, :],
                        rhs=xt[32 * st : 32 * st + 32, :],
                        start=True,
                        stop=True,
                        tile_position=(32 * st, 0),
                    )
                zq_v = zq.rearrange("p (s c) -> p s c", s=4)
                ua_v = ua.rearrange("p (s c) -> p s c", s=4)[:, :, 0:512]
                nc.vector.tensor_scalar(
                    out=ua_v,
                    in0=zq_v,
                    scalar1=bin2,
                    scalar2=0.0,
                    op0=mybir.AluOpType.add,
                    op1=mybir.AluOpType.max,
                )
                return zq, ua

            def emit_level(o, zq, ua, hprev, d):
                off = d - 1
                for p in range(4):
                    nc.tensor.matmul(
                        out=zq[:, 512 * p : 512 * p + TW],
                        lhsT=wxh2,
                        rhs=ua[:, USTRIDE * p + off : USTRIDE * p + off + TW],
                        start=True,
                        stop=(d == 1),
                    )
                    if d > 1:
                        nc.tensor.matmul(
                            out=zq[:, 512 * p : 512 * p + TW],
                            lhsT=whh2,
                            rhs=hprev[:, 512 * p : 512 * p + TW],
                            start=False,
                            stop=True,
                        )
                h = hpool.tile([128, 2048], f32r, tag="h", name=f"h{o}_{d}")
                nc.scalar.activation(out=h, in_=zq[:, :], func=Tanh, bias=bh2)
                return h

            def emit_y(o, zq, h4):
                for p in range(4):
                    nc.tensor.matmul(
                        out=zq[0:8, 0:TW],
                        lhsT=woutv[:, p, :],
                        rhs=h4[:, 512 * p : 512 * p + TW],
                        start=(p == 0),
                        stop=(p == 3),
                    )
                ysb = ypool.tile([8, TW], f32, tag="ysb", name=f"ysb{o}")
                nc.vector.tensor_scalar(
                    out=ysb,
                    in0=zq[0:8, 0:TW],
                    scalar1=bout[0:8],
                    scalar2=0.0,
                    op0=mybir.AluOpType.add,
                    op1=mybir.AluOpType.add,
                )
                nc.sync.dma_start(out=y_d[:, o, :], in_=ysb)

            # Octet pairs emitted in lockstep so one octet's matmuls run
            # under the other's tanh (ACT is the bottleneck engine).
            for oa in range(0, NOCT, 2):
                ob = oa + 1
                zqa, uaa = emit_u_phase(oa)
                zqb, uab = emit_u_phase(ob)
                ha = hb = None
                for d in range(1, 5):
                    ha = emit_level(oa, zqa, uaa, ha, d)
                    hb = emit_level(ob, zqb, uab, hb, d)
                emit_y(oa, zqa, ha)
                emit_y(ob, zqb, hb)

        sched_state, _ = tc.schedule_and_allocate()
        try:
            disp = sched_state.get_inst_dispatch_ns()
            vals = list(disp.values()) if hasattr(disp, "values") else list(disp)
            nc._pred_ns = max(t.dispatch_time_ns + t.cost_ns for t in vals) if vals else None
            busy = {}
            for t in vals:
                busy[str(t.engine)] = busy.get(str(t.engine), 0.0) + t.cost_ns
            nc._pred_busy = busy
            nc._sched_disp = disp
        except Exception as e:  # best-effort metric only
            nc._pred_ns = None
            nc._pred_err = repr(e)
    finally:
        tc_cm.__exit__(None, None, None)

    nc.finalize()
    return nc


def _get_nc():
    global _NC_CACHE
    if _NC_CACHE is None:
        _NC_CACHE = _build_nc()
    return _NC_CACHE


def _prep_weights(W_in, b_in, W_xh, W_hh, b_h, W_out, b_out):
    wu4 = np.zeros((128, 128), np.float32)
    for s in range(4):
        wu4[32 * s : 32 * s + 16, 0:64] = W_in
        wu4[32 * s + 16 : 32 * s + 32, 64:128] = W_in
    wxh2 = np.zeros((128, 128), np.float32)
    wxh2[0:64, 0:64] = W_xh
    wxh2[64:128, 64:128] = W_xh
    whh2 = np.zeros((128, 128), np.float32)
    whh2[0:64, 0:64] = W_hh
    whh2[64:128, 64:128] = W_hh
    woutv = np.zeros((128, 4, 8), np.float32)
    for p in range(4):
        woutv[0:64, p, 2 * p] = W_out[:, 0]
        woutv[64:128, p, 2 * p + 1] = W_out[:, 0]
    bin2 = np.concatenate([b_in, b_in]).reshape(128, 1).astype(np.float32)
    bh2 = np.concatenate([b_h, b_h]).reshape(128, 1).astype(np.float32)
    bout = np.full((128, 1), np.float32(b_out[0]), np.float32)
    return dict(wu4=wu4, wxh2=wxh2, whh2=whh2, woutv=woutv, bin2=bin2, bh2=bh2, bout=bout)


def _prep_x_core(x_c):
    # x_c: [NT, G_CORE, NX] -> xr [128, NOCT, TPAD] with
    # xr[16*b + f, o, t] = x_c[t, 8*o + b, f]
    t = x_c.transpose(1, 2, 0)                   # [g, f, t]
    t = t.reshape(NOCT, 8, NX, NT)               # [o, b, f, t]
    t = np.ascontiguousarray(t.transpose(1, 2, 0, 3).reshape(128, NOCT, NT))
    return t


def kernel(x, W_in, b_in, W_xh, W_hh, b_h, W_out, b_out, _trace=False):
    nc = _get_nc()
    w = _prep_weights(
        np.asarray(W_in, np.float32), np.asarray(b_in, np.float32),
        np.asarray(W_xh, np.float32), np.asarray(W_hh, np.float32),
        np.asarray(b_h, np.float32), np.asarray(W_out, np.float32),
        np.asarray(b_out, np.float32),
    )
    x = np.asarray(x, np.float32)
    in_maps = []
    for c in range(N_CORES):
        x_c = x[:, c * G_CORE : (c + 1) * G_CORE, :]
        in_maps.append({"xr": _prep_x_core(x_c), **w})

    res = run_bass_kernel_spmd(nc, in_maps, list(range(N_CORES)), trace=_trace)

    out = np.zeros((NT, NGRID, NY), np.float32)
    T = NT - D
    for c in range(N_CORES):
        y_c = res.results[c]["y"]                # [8, NOCT, TW]
        y_c = y_c[:, :, :T]                      # drop garbage windows
        y_c = y_c.transpose(2, 1, 0).reshape(T, G_CORE)  # [t, g]
        out[D:, c * G_CORE : (c + 1) * G_CORE, 0] = y_c
    if _trace:
        kernel.last_result = res
    return out
